# revision 6
# baseline (speedup 1.0000x reference)
"""Disparity estimation loss kernel for Trainium2 (Bass/Tile), 8-core SPMD.

Reference computation (per pixel over the D=192 disparity axis):
    prob    = softmax(cost_volume, axis=D)
    mean    = sum(prob * d)
    var     = sum(prob * (d - mean)^2) = E[d^2] - mean^2
    logvar  = log(var + 1e-6)
Outputs: (mean [B,H,W], logvar [B,H,W]) both f32.

Strategy: shard H across 8 cores (H=256 -> 32 rows/core). All reductions are
along D which stays local. Per core:
  - DMA cost volume with D on partitions: tiles [128, 4*512] (d-chunk 0..127
    for 4 h-rows) and a packed tile for d-chunk 128..191 of 8 h-rows.
  - exp on ScalarE -> bf16 (no max subtraction: inputs are N(0,1)).
  - TensorE matmuls contract over D: exp tile [D, 128 w-cols] stationary
    (bf16 -> fast weight load), weight columns [1, d, d^2_hi, d^2_lo] moving
    -> PSUM groups [128 w, 4]. d^2 split into exact-bf16 hi/lo bytes.
  - VectorE batched finalize (mean/var), ScalarE Ln, PE transpose, DMA out.
"""

import os
import sys

for _p in ("/opt/trn_rl_repo", "/root/.axon_site/_ro/trn_rl_repo"):
    if os.path.isdir(_p) and _p not in sys.path:
        sys.path.insert(0, _p)

import ml_dtypes
import numpy as np

import concourse.bacc as bacc
import concourse.bass as bass
import concourse.tile as tile
from concourse import mybir
from concourse.bass_utils import run_bass_kernel_spmd
from concourse.masks import make_identity

B, D, H, W = 4, 192, 256, 512
N_CORES = 8
HL = H // N_CORES  # 32 h-rows per core
F32 = mybir.dt.float32
BF16 = mybir.dt.bfloat16

# knobs (test.py may flip these before calling kernel())
TRACE = False
LAST_RESULT = None


def _make_weights() -> np.ndarray:
    """[128, 12] bf16 weight matrix; every entry is exactly representable.

    cols 0:4  -> d-chunk0 (d = row p):        [1, d, hi(d^2), lo(d^2)]
    cols 4:12 -> packed d-chunk1 (two slabs stacked on partitions):
       rows 0:64   (slab lo, d = 128+p):      [1, d, hi, lo, 0, 0, 0, 0]
       rows 64:128 (slab hi, d = 64+p):       [0, 0, 0, 0, 1, d, hi, lo]
    where hi = d^2 >> 8 (<=142), lo = d^2 & 255 — both exact in bf16.
    """
    wk = np.zeros((128, 12), dtype=np.float64)

    def cols(d):
        dsq = (d.astype(np.int64)) ** 2
        return 1.0, d, (dsq >> 8).astype(np.float64), (dsq & 255).astype(np.float64)

    p = np.arange(128, dtype=np.int64)
    wk[:, 0], wk[:, 1], wk[:, 2], wk[:, 3] = cols(p)
    c = cols(128 + p[:64])
    for k in range(4):
        wk[:64, 4 + k] = c[k]
    c = cols(64 + p[64:])
    for k in range(4):
        wk[64:, 8 + k] = c[k]
    return wk.astype(ml_dtypes.bfloat16)


def build_core_kernel():
    """Build the per-core Bass module (identical program on all 8 cores)."""
    nc = bacc.Bacc("TRN2", target_bir_lowering=False, debug=False)
    x = nc.dram_tensor("x", [B, D, HL, W], F32, kind="ExternalInput")
    wk = nc.dram_tensor("wk", [128, 12], BF16, kind="ExternalInput")
    mean_o = nc.dram_tensor("mean", [B, HL, W], F32, kind="ExternalOutput")
    logv_o = nc.dram_tensor("logvar", [B, HL, W], F32, kind="ExternalOutput")

    with tile.TileContext(nc) as tc:
        with (
            tc.tile_pool(name="cv", bufs=3) as cvp,
            tc.tile_pool(name="ex", bufs=3) as exp_p,
            tc.tile_pool(name="consts", bufs=1) as consts,
            tc.tile_pool(name="fin", bufs=2) as finp,
            tc.tile_pool(name="tmps", bufs=2) as tmpp,
            tc.tile_pool(name="outp", bufs=2) as outp,
            tc.tile_pool(name="psum", bufs=2, space="PSUM") as psp,
            tc.tile_pool(name="pst", bufs=2, space="PSUM") as pstp,
        ):
            wkt = consts.tile([128, 12], BF16, tag="wk")
            nc.sync.dma_start(out=wkt, in_=wk[:, :])
            ident = consts.tile([128, 128], F32, tag="ident")
            make_identity(nc, ident)
            eps_t = consts.tile([128, 1], F32, tag="eps")
            nc.vector.memset(eps_t, 1e-6)

            for b in range(B):
                # one PSUM bank per b for chunk0 sums, one for chunk1 sums
                bankA = psp.tile([128, 512], F32, tag="bankA")
                bankB = psp.tile([128, 512], F32, tag="bankB")

                for g in range(4):  # supergroup: h rows 8g .. 8g+7
                    h0 = 8 * g
                    lo = cvp.tile([128, 4 * W], F32, tag="lo")
                    hi = cvp.tile([128, 4 * W], F32, tag="hi")
                    c1 = cvp.tile([128, 4 * W], F32, tag="c1")
                    # d 0..127 for h rows h0..h0+3 / h0+4..h0+7
                    nc.sync.dma_start(out=lo, in_=x[b, 0:128, h0 : h0 + 4, :])
                    nc.sync.dma_start(out=hi, in_=x[b, 0:128, h0 + 4 : h0 + 8, :])
                    # d 128..191 for all 8 h rows, packed on partitions:
                    # partitions 0:64 = h rows h0..h0+3, 64:128 = h0+4..h0+7
                    nc.sync.dma_start(
                        out=c1,
                        in_=x[b, 128:192, h0 : h0 + 8, :].rearrange(
                            "d (p h) w -> p d h w", p=2
                        ),
                    )
                    # exp -> bf16 (fast weight load for the matmuls)
                    elo = exp_p.tile([128, 4 * W], BF16, tag="elo")
                    ehi = exp_p.tile([128, 4 * W], BF16, tag="ehi")
                    ec1 = exp_p.tile([128, 4 * W], BF16, tag="ec1")
                    for src, dst in ((lo, elo), (hi, ehi), (c1, ec1)):
                        nc.scalar.activation(
                            out=dst, in_=src, func=mybir.ActivationFunctionType.Exp
                        )
                    # matmuls: contract over D. All are singleton accumulation
                    # groups into disjoint PSUM columns (no has_written games).
                    for i in range(4):  # h row within group
                        for wc in range(4):  # 128-col W chunk
                            j2 = g * 16 + i * 4 + wc
                            off = 8 * j2
                            sl = slice(i * W + wc * 128, i * W + wc * 128 + 128)
                            # chunk1 (d 128..191), both slabs at once: N=8
                            nc.tensor.matmul(
                                bankB[:, off : off + 8],
                                ec1[:, sl],
                                wkt[:, 4:12],
                                start=True,
                                stop=True,
                            )
                            # chunk0 lo slab (h0+i): N=4 at cols off..off+3
                            nc.tensor.matmul(
                                bankA[:, off : off + 4],
                                elo[:, sl],
                                wkt[:, 0:4],
                                start=True,
                                stop=True,
                            )
                            # chunk0 hi slab (h0+4+i): N=4 at cols off+4..off+7
                            nc.tensor.matmul(
                                bankA[:, off + 4 : off + 8],
                                ehi[:, sl],
                                wkt[:, 0:4],
                                start=True,
                                stop=True,
                            )

                # ---- finalize whole b: mean/var on [128 w, 128 j3] tiles ----
                # TensorTensor may read only one PSUM operand: evacuate bankB
                # to SBUF first, then adds read (PSUM bankA, SBUF copy).
                bB_sb = tmpp.tile([128, 512], F32, tag="bB_sb")
                nc.vector.tensor_copy(bB_sb, bankB)
                # views: [128, g:4, i:4, wc:4] at col 8*(16g+4i+wc) + e
                A5 = bankA.rearrange("p (g i w e) -> p g i w e", g=4, i=4, w=4)
                B5 = bB_sb.rearrange("p (g i w e) -> p g i w e", g=4, i=4, w=4)
                mean_sb = finp.tile([128, 128], F32, tag="mean_sb")
                var_sb = finp.tile([128, 128], F32, tag="var_sb")
                # dest col j3 = 32g + 16*half + 4i + wc  (h = 8g+4*half+i)
                M5 = mean_sb.rearrange("p (g f i w) -> p g f i w", g=4, f=2, i=4)
                V5 = var_sb.rearrange("p (g f i w) -> p g f i w", g=4, f=2, i=4)

                for half in range(2):  # 0 = lo slabs, 1 = hi slabs
                    so = 4 * half
                    s0t = tmpp.tile([128, 4, 4, 4], F32, tag="s0t")
                    s1t = tmpp.tile([128, 4, 4, 4], F32, tag="s1t")
                    s2h = tmpp.tile([128, 4, 4, 4], F32, tag="s2h")
                    s2t = tmpp.tile([128, 4, 4, 4], F32, tag="s2t")
                    rt = tmpp.tile([128, 4, 4, 4], F32, tag="rt")
                    m2t = tmpp.tile([128, 4, 4, 4], F32, tag="m2t")
                    msqt = tmpp.tile([128, 4, 4, 4], F32, tag="msqt")
                    nc.vector.tensor_add(
                        s0t, A5[:, :, :, :, so + 0], B5[:, :, :, :, so + 0]
                    )
                    nc.vector.tensor_add(
                        s1t, A5[:, :, :, :, so + 1], B5[:, :, :, :, so + 1]
                    )
                    nc.vector.tensor_add(
                        s2h, A5[:, :, :, :, so + 2], B5[:, :, :, :, so + 2]
                    )
                    nc.vector.tensor_add(
                        s2t, A5[:, :, :, :, so + 3], B5[:, :, :, :, so + 3]
                    )
                    # s2 = 256*hi + lo
                    nc.vector.scalar_tensor_tensor(
                        out=s2t,
                        in0=s2h,
                        scalar=256.0,
                        in1=s2t,
                        op0=mybir.AluOpType.mult,
                        op1=mybir.AluOpType.add,
                    )
                    nc.vector.reciprocal(rt, s0t)
                    mv = M5[:, :, half, :, :]
                    nc.vector.tensor_mul(mv, s1t, rt)  # mean = s1/s0
                    nc.vector.tensor_mul(m2t, s2t, rt)  # E[d^2]
                    nc.vector.tensor_mul(msqt, mv, mv)  # mean^2
                    nc.vector.tensor_sub(V5[:, :, half, :, :], m2t, msqt)

                # transpose [w, j3] -> [j3, w] and write out
                mt_ps = pstp.tile([128, 128], F32, tag="tp")
                nc.tensor.transpose(mt_ps, mean_sb, ident)
                mo_sb = outp.tile([128, 128], F32, tag="mo")
                nc.vector.tensor_copy(mo_sb, mt_ps)
                vt_ps = pstp.tile([128, 128], F32, tag="tp")
                nc.tensor.transpose(vt_ps, var_sb, ident)
                lo_sb = outp.tile([128, 128], F32, tag="lv")
                nc.scalar.activation(
                    out=lo_sb,
                    in_=vt_ps,
                    func=mybir.ActivationFunctionType.Ln,
                    bias=eps_t,
                    scale=1.0,
                )
                # partition j3 = 4h + wc; free = w (512B contiguous rows)
                nc.sync.dma_start(
                    out=mean_o[b].rearrange("h (c w) -> h c w", c=4), in_=mo_sb
                )
                nc.sync.dma_start(
                    out=logv_o[b].rearrange("h (c w) -> h c w", c=4), in_=lo_sb
                )

    nc.compile()
    return nc


_NC_CACHE = None


def _get_nc():
    global _NC_CACHE
    if _NC_CACHE is None:
        _NC_CACHE = build_core_kernel()
    return _NC_CACHE


def kernel(cost_volume: np.ndarray):
    global LAST_RESULT
    cost_volume = np.ascontiguousarray(np.asarray(cost_volume, dtype=np.float32))
    assert cost_volume.shape == (B, D, H, W), cost_volume.shape

    nc = _get_nc()
    wk = _make_weights()
    in_maps = []
    for c in range(N_CORES):
        shard = np.ascontiguousarray(cost_volume[:, :, c * HL : (c + 1) * HL, :])
        in_maps.append({"x": shard, "wk": wk})

    res = run_bass_kernel_spmd(nc, in_maps, list(range(N_CORES)), trace=TRACE)
    LAST_RESULT = res

    mean = np.empty((B, H, W), dtype=np.float32)
    logv = np.empty((B, H, W), dtype=np.float32)
    for c in range(N_CORES):
        mean[:, c * HL : (c + 1) * HL, :] = res.results[c]["mean"]
        logv[:, c * HL : (c + 1) * HL, :] = res.results[c]["logvar"]
    return mean, logv


# revision 8
# speedup vs baseline: 1.5569x; 1.5569x over previous
"""Disparity estimation loss kernel for Trainium2 (Bass/Tile), 8-core SPMD.

Reference computation (per pixel over the D=192 disparity axis):
    prob    = softmax(cost_volume, axis=D)
    mean    = sum(prob * d)
    var     = sum(prob * (d - mean)^2) = E[d^2] - mean^2
    logvar  = log(var + 1e-6)
Outputs: (mean [B,H,W], logvar [B,H,W]) both f32.

Strategy: shard H across 8 cores (H=256 -> 32 rows/core). All reductions are
along D which stays local. Per core:
  - DMA cost volume with D on partitions: tiles [128, 4*512] (d-chunk 0..127
    for 4 h-rows) and a packed tile for d-chunk 128..191 of 8 h-rows.
  - exp on ScalarE -> bf16 (no max subtraction: inputs are N(0,1)).
  - TensorE matmuls contract over D: exp tile [D, 128 w-cols] stationary
    (bf16 -> fast weight load), weight columns [1, d, d^2_hi, d^2_lo] moving
    -> PSUM groups [128 w, 4]. d^2 split into exact-bf16 hi/lo bytes.
  - VectorE batched finalize (mean/var), ScalarE Ln, PE transpose, DMA out.
"""

import os
import sys

for _p in ("/opt/trn_rl_repo", "/root/.axon_site/_ro/trn_rl_repo"):
    if os.path.isdir(_p) and _p not in sys.path:
        sys.path.insert(0, _p)

import ml_dtypes
import numpy as np

import concourse.bacc as bacc
import concourse.bass as bass
import concourse.tile as tile
from concourse import mybir
from concourse.bass_utils import run_bass_kernel_spmd
from concourse.masks import make_identity

B, D, H, W = 4, 192, 256, 512
N_CORES = 8
HL = H // N_CORES  # 32 h-rows per core
F32 = mybir.dt.float32
BF16 = mybir.dt.bfloat16

# knobs (test.py may flip these before calling kernel())
TRACE = False
LAST_RESULT = None


def _make_weights() -> np.ndarray:
    """[128, 12] bf16 weight matrix; every entry is exactly representable.

    cols 0:4  -> d-chunk0 (d = row p):        [1, d, hi(d^2), lo(d^2)]
    cols 4:12 -> packed d-chunk1 (two slabs stacked on partitions):
       rows 0:64   (slab lo, d = 128+p):      [1, d, hi, lo, 0, 0, 0, 0]
       rows 64:128 (slab hi, d = 64+p):       [0, 0, 0, 0, 1, d, hi, lo]
    where hi = d^2 >> 8 (<=142), lo = d^2 & 255 — both exact in bf16.
    """
    wk = np.zeros((128, 12), dtype=np.float64)

    def cols(d):
        dsq = (d.astype(np.int64)) ** 2
        return 1.0, d, (dsq >> 8).astype(np.float64), (dsq & 255).astype(np.float64)

    p = np.arange(128, dtype=np.int64)
    wk[:, 0], wk[:, 1], wk[:, 2], wk[:, 3] = cols(p)
    c = cols(128 + p[:64])
    for k in range(4):
        wk[:64, 4 + k] = c[k]
    c = cols(64 + p[64:])
    for k in range(4):
        wk[64:, 8 + k] = c[k]
    return wk.astype(ml_dtypes.bfloat16)


def build_core_kernel():
    """Build the per-core Bass module (identical program on all 8 cores)."""
    nc = bacc.Bacc("TRN2", target_bir_lowering=False, debug=False)
    x = nc.dram_tensor("x", [B, D, HL, W], F32, kind="ExternalInput")
    wk = nc.dram_tensor("wk", [128, 12], BF16, kind="ExternalInput")
    mean_o = nc.dram_tensor("mean", [B, HL, W], F32, kind="ExternalOutput")
    logv_o = nc.dram_tensor("logvar", [B, HL, W], F32, kind="ExternalOutput")

    with tile.TileContext(nc) as tc:
        with (
            tc.tile_pool(name="cv", bufs=3) as cvp,
            tc.tile_pool(name="ex", bufs=3) as exp_p,
            tc.tile_pool(name="consts", bufs=1) as consts,
            tc.tile_pool(name="fin", bufs=2) as finp,
            tc.tile_pool(name="tmps", bufs=2) as tmpp,
            tc.tile_pool(name="outp", bufs=2) as outp,
            tc.tile_pool(name="psum", bufs=2, space="PSUM") as psp,
            tc.tile_pool(name="pst", bufs=2, space="PSUM") as pstp,
        ):
            wkt = consts.tile([128, 12], BF16, tag="wk")
            nc.sync.dma_start(out=wkt, in_=wk[:, :])
            ident = consts.tile([128, 128], F32, tag="ident")
            make_identity(nc, ident)
            eps_t = consts.tile([128, 1], F32, tag="eps")
            nc.vector.memset(eps_t, 1e-6)

            for b in range(B):
                # one PSUM bank per b for chunk0 sums, one for chunk1 sums
                bankA = psp.tile([128, 512], F32, tag="bankA")
                bankB = psp.tile([128, 512], F32, tag="bankB")

                for g in range(4):  # supergroup: h rows 8g .. 8g+7
                    h0 = 8 * g
                    cv0 = cvp.tile([128, 8 * W], F32, tag="cv0")
                    c1 = cvp.tile([128, 4 * W], F32, tag="c1")
                    # d 0..127 for h rows h0..h0+7 (2 MiB, 16KB/partition).
                    # Alternate the two HWDGE rings; c1 rides the SWDGE ring —
                    # three independent queues so transfers overlap.
                    eng0 = nc.sync if g % 2 == 0 else nc.scalar
                    eng0.dma_start(out=cv0, in_=x[b, 0:128, h0 : h0 + 8, :])
                    # d 128..191 for all 8 h rows, packed on partitions:
                    # partitions 0:64 = h rows h0..h0+3, 64:128 = h0+4..h0+7
                    nc.gpsimd.dma_start(
                        out=c1,
                        in_=x[b, 128:192, h0 : h0 + 8, :].rearrange(
                            "d (p h) w -> p d h w", p=2
                        ),
                    )
                    # exp -> bf16 (fast weight load for the matmuls)
                    ecv0 = exp_p.tile([128, 8 * W], BF16, tag="ecv0")
                    ec1 = exp_p.tile([128, 4 * W], BF16, tag="ec1")
                    for src, dst in ((cv0, ecv0), (c1, ec1)):
                        nc.scalar.activation(
                            out=dst, in_=src, func=mybir.ActivationFunctionType.Exp
                        )
                    # matmuls: contract over D. All are singleton accumulation
                    # groups into disjoint PSUM columns (no has_written games).
                    for i in range(4):  # h row within group
                        for wc in range(4):  # 128-col W chunk
                            j2 = g * 16 + i * 4 + wc
                            off = 8 * j2
                            sl = slice(i * W + wc * 128, i * W + wc * 128 + 128)
                            # chunk1 (d 128..191), both slabs at once: N=8
                            nc.tensor.matmul(
                                bankB[:, off : off + 8],
                                ec1[:, sl],
                                wkt[:, 4:12],
                                start=True,
                                stop=True,
                            )
                            # chunk0 lo slab (h0+i): N=4 at cols off..off+3
                            nc.tensor.matmul(
                                bankA[:, off : off + 4],
                                ecv0[:, sl],
                                wkt[:, 0:4],
                                start=True,
                                stop=True,
                            )
                            # chunk0 hi slab (h0+4+i): N=4 at cols off+4..off+7
                            sl_hi = slice(
                                (4 + i) * W + wc * 128, (4 + i) * W + wc * 128 + 128
                            )
                            nc.tensor.matmul(
                                bankA[:, off + 4 : off + 8],
                                ecv0[:, sl_hi],
                                wkt[:, 0:4],
                                start=True,
                                stop=True,
                            )

                # ---- finalize whole b: mean/var on [128 w, 128 j3] tiles ----
                # TensorTensor may read only one PSUM operand: evacuate bankB
                # to SBUF first, then adds read (PSUM bankA, SBUF copy).
                bB_sb = tmpp.tile([128, 512], F32, tag="bB_sb")
                nc.vector.tensor_copy(bB_sb, bankB)
                # views: [128, g:4, i:4, wc:4] at col 8*(16g+4i+wc) + e
                A5 = bankA.rearrange("p (g i w e) -> p g i w e", g=4, i=4, w=4)
                B5 = bB_sb.rearrange("p (g i w e) -> p g i w e", g=4, i=4, w=4)
                mean_sb = finp.tile([128, 128], F32, tag="mean_sb")
                var_sb = finp.tile([128, 128], F32, tag="var_sb")
                # dest col j3 = 32g + 16*half + 4i + wc  (h = 8g+4*half+i)
                M5 = mean_sb.rearrange("p (g f i w) -> p g f i w", g=4, f=2, i=4)
                V5 = var_sb.rearrange("p (g f i w) -> p g f i w", g=4, f=2, i=4)

                for half in range(2):  # 0 = lo slabs, 1 = hi slabs
                    so = 4 * half
                    s0t = tmpp.tile([128, 4, 4, 4], F32, tag="s0t")
                    s1t = tmpp.tile([128, 4, 4, 4], F32, tag="s1t")
                    s2h = tmpp.tile([128, 4, 4, 4], F32, tag="s2h")
                    s2t = tmpp.tile([128, 4, 4, 4], F32, tag="s2t")
                    rt = tmpp.tile([128, 4, 4, 4], F32, tag="rt")
                    m2t = tmpp.tile([128, 4, 4, 4], F32, tag="m2t")
                    msqt = tmpp.tile([128, 4, 4, 4], F32, tag="msqt")
                    nc.vector.tensor_add(
                        s0t, A5[:, :, :, :, so + 0], B5[:, :, :, :, so + 0]
                    )
                    nc.vector.tensor_add(
                        s1t, A5[:, :, :, :, so + 1], B5[:, :, :, :, so + 1]
                    )
                    nc.vector.tensor_add(
                        s2h, A5[:, :, :, :, so + 2], B5[:, :, :, :, so + 2]
                    )
                    nc.vector.tensor_add(
                        s2t, A5[:, :, :, :, so + 3], B5[:, :, :, :, so + 3]
                    )
                    # s2 = 256*hi + lo
                    nc.vector.scalar_tensor_tensor(
                        out=s2t,
                        in0=s2h,
                        scalar=256.0,
                        in1=s2t,
                        op0=mybir.AluOpType.mult,
                        op1=mybir.AluOpType.add,
                    )
                    nc.vector.reciprocal(rt, s0t)
                    mv = M5[:, :, half, :, :]
                    nc.vector.tensor_mul(mv, s1t, rt)  # mean = s1/s0
                    nc.vector.tensor_mul(m2t, s2t, rt)  # E[d^2]
                    nc.vector.tensor_mul(msqt, mv, mv)  # mean^2
                    nc.vector.tensor_sub(V5[:, :, half, :, :], m2t, msqt)

                # transpose [w, j3] -> [j3, w] and write out
                mt_ps = pstp.tile([128, 128], F32, tag="tp")
                nc.tensor.transpose(mt_ps, mean_sb, ident)
                mo_sb = outp.tile([128, 128], F32, tag="mo")
                nc.vector.tensor_copy(mo_sb, mt_ps)
                vt_ps = pstp.tile([128, 128], F32, tag="tp")
                nc.tensor.transpose(vt_ps, var_sb, ident)
                lo_sb = outp.tile([128, 128], F32, tag="lv")
                nc.scalar.activation(
                    out=lo_sb,
                    in_=vt_ps,
                    func=mybir.ActivationFunctionType.Ln,
                    bias=eps_t,
                    scale=1.0,
                )
                # partition j3 = 4h + wc; free = w (512B contiguous rows)
                nc.sync.dma_start(
                    out=mean_o[b].rearrange("h (c w) -> h c w", c=4), in_=mo_sb
                )
                nc.sync.dma_start(
                    out=logv_o[b].rearrange("h (c w) -> h c w", c=4), in_=lo_sb
                )

    nc.compile()
    return nc


_NC_CACHE = None


def _get_nc():
    global _NC_CACHE
    if _NC_CACHE is None:
        _NC_CACHE = build_core_kernel()
    return _NC_CACHE


def kernel(cost_volume: np.ndarray):
    global LAST_RESULT
    cost_volume = np.ascontiguousarray(np.asarray(cost_volume, dtype=np.float32))
    assert cost_volume.shape == (B, D, H, W), cost_volume.shape

    nc = _get_nc()
    wk = _make_weights()
    in_maps = []
    for c in range(N_CORES):
        shard = np.ascontiguousarray(cost_volume[:, :, c * HL : (c + 1) * HL, :])
        in_maps.append({"x": shard, "wk": wk})

    res = run_bass_kernel_spmd(nc, in_maps, list(range(N_CORES)), trace=TRACE)
    LAST_RESULT = res

    mean = np.empty((B, H, W), dtype=np.float32)
    logv = np.empty((B, H, W), dtype=np.float32)
    for c in range(N_CORES):
        mean[:, c * HL : (c + 1) * HL, :] = res.results[c]["mean"]
        logv[:, c * HL : (c + 1) * HL, :] = res.results[c]["logvar"]
    return mean, logv


# revision 11
# speedup vs baseline: 1.5872x; 1.0195x over previous
"""Disparity estimation loss kernel for Trainium2 (Bass/Tile), 8-core SPMD.

Reference computation (per pixel over the D=192 disparity axis):
    prob    = softmax(cost_volume, axis=D)
    mean    = sum(prob * d)
    var     = sum(prob * (d - mean)^2) = E[d^2] - mean^2
    logvar  = log(var + 1e-6)
Outputs: (mean [B,H,W], logvar [B,H,W]) both f32.

Strategy: shard H across 8 cores (H=256 -> 32 rows/core). All reductions are
along D which stays local. Per core:
  - DMA cost volume with D on partitions: tiles [128, 4*512] (d-chunk 0..127
    for 4 h-rows) and a packed tile for d-chunk 128..191 of 8 h-rows.
  - exp on ScalarE -> bf16 (no max subtraction: inputs are N(0,1)).
  - TensorE matmuls contract over D: exp tile [D, 128 w-cols] stationary
    (bf16 -> fast weight load), weight columns [1, d, d^2_hi, d^2_lo] moving
    -> PSUM groups [128 w, 4]. d^2 split into exact-bf16 hi/lo bytes.
  - VectorE batched finalize (mean/var), ScalarE Ln, PE transpose, DMA out.
"""

import os
import sys

for _p in ("/opt/trn_rl_repo", "/root/.axon_site/_ro/trn_rl_repo"):
    if os.path.isdir(_p) and _p not in sys.path:
        sys.path.insert(0, _p)

import ml_dtypes
import numpy as np

import concourse.bacc as bacc
import concourse.bass as bass
import concourse.tile as tile
from concourse import mybir
from concourse.bass_utils import run_bass_kernel_spmd
from concourse.masks import make_identity

B, D, H, W = 4, 192, 256, 512
N_CORES = 8
HL = H // N_CORES  # 32 h-rows per core
F32 = mybir.dt.float32
BF16 = mybir.dt.bfloat16

# knobs (test.py may flip these before calling kernel())
TRACE = False
LAST_RESULT = None


def _make_weights() -> np.ndarray:
    """[128, 12] bf16 weight matrix; every entry is exactly representable.

    cols 0:4  -> d-chunk0 (d = row p):        [1, d, hi(d^2), lo(d^2)]
    cols 4:12 -> packed d-chunk1 (two slabs stacked on partitions):
       rows 0:64   (slab lo, d = 128+p):      [1, d, hi, lo, 0, 0, 0, 0]
       rows 64:128 (slab hi, d = 64+p):       [0, 0, 0, 0, 1, d, hi, lo]
    where hi = d^2 >> 8 (<=142), lo = d^2 & 255 — both exact in bf16.
    """
    wk = np.zeros((128, 12), dtype=np.float64)

    def cols(d):
        dsq = (d.astype(np.int64)) ** 2
        return 1.0, d, (dsq >> 8).astype(np.float64), (dsq & 255).astype(np.float64)

    p = np.arange(128, dtype=np.int64)
    wk[:, 0], wk[:, 1], wk[:, 2], wk[:, 3] = cols(p)
    c = cols(128 + p[:64])
    for k in range(4):
        wk[:64, 4 + k] = c[k]
    c = cols(64 + p[64:])
    for k in range(4):
        wk[64:, 8 + k] = c[k]
    return wk.astype(ml_dtypes.bfloat16)


def build_core_kernel():
    """Build the per-core Bass module (identical program on all 8 cores)."""
    nc = bacc.Bacc("TRN2", target_bir_lowering=False, debug=False)
    x = nc.dram_tensor("x", [B, D, HL, W], F32, kind="ExternalInput")
    wk = nc.dram_tensor("wk", [128, 12], BF16, kind="ExternalInput")
    mean_o = nc.dram_tensor("mean", [B, HL, W], F32, kind="ExternalOutput")
    logv_o = nc.dram_tensor("logvar", [B, HL, W], F32, kind="ExternalOutput")

    with tile.TileContext(nc) as tc:
        with (
            tc.tile_pool(name="cv", bufs=4) as cvp,
            tc.tile_pool(name="ex", bufs=4) as exp_p,
            tc.tile_pool(name="consts", bufs=1) as consts,
            tc.tile_pool(name="fin", bufs=2) as finp,
            tc.tile_pool(name="tmps", bufs=2) as tmpp,
            tc.tile_pool(name="outp", bufs=2) as outp,
            tc.tile_pool(name="psum", bufs=3, space="PSUM") as psp,
            tc.tile_pool(name="pst", bufs=2, space="PSUM") as pstp,
        ):
            wkt = consts.tile([128, 12], BF16, tag="wk")
            nc.sync.dma_start(out=wkt, in_=wk[:, :])
            ident = consts.tile([128, 128], F32, tag="ident")
            make_identity(nc, ident)
            eps_t = consts.tile([128, 1], F32, tag="eps")
            nc.vector.memset(eps_t, 1e-6)

            for b in range(B):
                # one PSUM bank per b for chunk0 sums, one for chunk1 sums
                bankA = psp.tile([128, 512], F32, tag="bankA")
                bankB = psp.tile([128, 512], F32, tag="bankB")

                for g in range(4):  # supergroup: h rows 8g .. 8g+7
                    h0 = 8 * g
                    cv0 = cvp.tile([128, 8 * W], F32, tag="cv0")
                    c1 = cvp.tile([128, 4 * W], F32, tag="c1")
                    # d 0..127 for h rows h0..h0+7 (2 MiB, 16KB/partition),
                    # split halves across the two HWDGE rings; c1 rides the
                    # SWDGE ring — three queues stream concurrently.
                    nc.sync.dma_start(
                        out=cv0[:, 0 : 4 * W], in_=x[b, 0:128, h0 : h0 + 4, :]
                    )
                    nc.scalar.dma_start(
                        out=cv0[:, 4 * W : 8 * W],
                        in_=x[b, 0:128, h0 + 4 : h0 + 8, :],
                    )
                    # d 128..191 for all 8 h rows, packed on partitions:
                    # partitions 0:64 = h rows h0..h0+3, 64:128 = h0+4..h0+7
                    nc.gpsimd.dma_start(
                        out=c1,
                        in_=x[b, 128:192, h0 : h0 + 8, :].rearrange(
                            "d (p h) w -> p d h w", p=2
                        ),
                    )
                    # exp -> bf16 (fast weight load for the matmuls)
                    ecv0 = exp_p.tile([128, 8 * W], BF16, tag="ecv0")
                    ec1 = exp_p.tile([128, 4 * W], BF16, tag="ec1")
                    for src, dst in ((cv0, ecv0), (c1, ec1)):
                        nc.scalar.activation(
                            out=dst, in_=src, func=mybir.ActivationFunctionType.Exp
                        )
                    # matmuls: contract over D. All are singleton accumulation
                    # groups into disjoint PSUM columns (no has_written games).
                    for i in range(4):  # h row within group
                        for wc in range(4):  # 128-col W chunk
                            j2 = g * 16 + i * 4 + wc
                            off = 8 * j2
                            sl = slice(i * W + wc * 128, i * W + wc * 128 + 128)
                            # chunk1 (d 128..191), both slabs at once: N=8
                            nc.tensor.matmul(
                                bankB[:, off : off + 8],
                                ec1[:, sl],
                                wkt[:, 4:12],
                                start=True,
                                stop=True,
                            )
                            # chunk0 lo slab (h0+i): N=4 at cols off..off+3
                            nc.tensor.matmul(
                                bankA[:, off : off + 4],
                                ecv0[:, sl],
                                wkt[:, 0:4],
                                start=True,
                                stop=True,
                            )
                            # chunk0 hi slab (h0+4+i): N=4 at cols off+4..off+7
                            sl_hi = slice(
                                (4 + i) * W + wc * 128, (4 + i) * W + wc * 128 + 128
                            )
                            nc.tensor.matmul(
                                bankA[:, off + 4 : off + 8],
                                ecv0[:, sl_hi],
                                wkt[:, 0:4],
                                start=True,
                                stop=True,
                            )

                # ---- finalize whole b: mean/var on [128 w, 128 j3] tiles ----
                # TensorTensor may read only one PSUM operand: evacuate bankB
                # to SBUF first, then adds read (PSUM bankA, SBUF copy).
                bB_sb = tmpp.tile([128, 512], F32, tag="bB_sb")
                nc.vector.tensor_copy(bB_sb, bankB)
                # views: [128, g:4, i:4, wc:4] at col 8*(16g+4i+wc) + e
                A5 = bankA.rearrange("p (g i w e) -> p g i w e", g=4, i=4, w=4)
                B5 = bB_sb.rearrange("p (g i w e) -> p g i w e", g=4, i=4, w=4)
                mean_sb = finp.tile([128, 128], F32, tag="mean_sb")
                var_sb = finp.tile([128, 128], F32, tag="var_sb")
                # dest col j3 = 32g + 16*half + 4i + wc  (h = 8g+4*half+i)
                M5 = mean_sb.rearrange("p (g f i w) -> p g f i w", g=4, f=2, i=4)
                V5 = var_sb.rearrange("p (g f i w) -> p g f i w", g=4, f=2, i=4)

                for half in range(2):  # 0 = lo slabs, 1 = hi slabs
                    so = 4 * half
                    s0t = tmpp.tile([128, 4, 4, 4], F32, tag="s0t")
                    s1t = tmpp.tile([128, 4, 4, 4], F32, tag="s1t")
                    s2h = tmpp.tile([128, 4, 4, 4], F32, tag="s2h")
                    s2t = tmpp.tile([128, 4, 4, 4], F32, tag="s2t")
                    rt = tmpp.tile([128, 4, 4, 4], F32, tag="rt")
                    m2t = tmpp.tile([128, 4, 4, 4], F32, tag="m2t")
                    msqt = tmpp.tile([128, 4, 4, 4], F32, tag="msqt")
                    nc.vector.tensor_add(
                        s0t, A5[:, :, :, :, so + 0], B5[:, :, :, :, so + 0]
                    )
                    nc.vector.tensor_add(
                        s1t, A5[:, :, :, :, so + 1], B5[:, :, :, :, so + 1]
                    )
                    nc.vector.tensor_add(
                        s2h, A5[:, :, :, :, so + 2], B5[:, :, :, :, so + 2]
                    )
                    nc.vector.tensor_add(
                        s2t, A5[:, :, :, :, so + 3], B5[:, :, :, :, so + 3]
                    )
                    # s2 = 256*hi + lo
                    nc.vector.scalar_tensor_tensor(
                        out=s2t,
                        in0=s2h,
                        scalar=256.0,
                        in1=s2t,
                        op0=mybir.AluOpType.mult,
                        op1=mybir.AluOpType.add,
                    )
                    nc.vector.reciprocal(rt, s0t)
                    mv = M5[:, :, half, :, :]
                    nc.vector.tensor_mul(mv, s1t, rt)  # mean = s1/s0
                    nc.vector.tensor_mul(m2t, s2t, rt)  # E[d^2]
                    nc.vector.tensor_mul(msqt, mv, mv)  # mean^2
                    nc.vector.tensor_sub(V5[:, :, half, :, :], m2t, msqt)

                # transpose [w, j3] -> [j3, w] and write out
                mt_ps = pstp.tile([128, 128], F32, tag="tp")
                nc.tensor.transpose(mt_ps, mean_sb, ident)
                mo_sb = outp.tile([128, 128], F32, tag="mo")
                nc.vector.tensor_copy(mo_sb, mt_ps)
                vt_ps = pstp.tile([128, 128], F32, tag="tp")
                nc.tensor.transpose(vt_ps, var_sb, ident)
                lo_sb = outp.tile([128, 128], F32, tag="lv")
                nc.scalar.activation(
                    out=lo_sb,
                    in_=vt_ps,
                    func=mybir.ActivationFunctionType.Ln,
                    bias=eps_t,
                    scale=1.0,
                )
                # partition j3 = 4h + wc; free = w (512B contiguous rows)
                nc.sync.dma_start(
                    out=mean_o[b].rearrange("h (c w) -> h c w", c=4), in_=mo_sb
                )
                nc.sync.dma_start(
                    out=logv_o[b].rearrange("h (c w) -> h c w", c=4), in_=lo_sb
                )

    nc.compile()
    return nc


_NC_CACHE = None


def _get_nc():
    global _NC_CACHE
    if _NC_CACHE is None:
        _NC_CACHE = build_core_kernel()
    return _NC_CACHE


def kernel(cost_volume: np.ndarray):
    global LAST_RESULT
    cost_volume = np.ascontiguousarray(np.asarray(cost_volume, dtype=np.float32))
    assert cost_volume.shape == (B, D, H, W), cost_volume.shape

    nc = _get_nc()
    wk = _make_weights()
    in_maps = []
    for c in range(N_CORES):
        shard = np.ascontiguousarray(cost_volume[:, :, c * HL : (c + 1) * HL, :])
        in_maps.append({"x": shard, "wk": wk})

    res = run_bass_kernel_spmd(nc, in_maps, list(range(N_CORES)), trace=TRACE)
    LAST_RESULT = res

    mean = np.empty((B, H, W), dtype=np.float32)
    logv = np.empty((B, H, W), dtype=np.float32)
    for c in range(N_CORES):
        mean[:, c * HL : (c + 1) * HL, :] = res.results[c]["mean"]
        logv[:, c * HL : (c + 1) * HL, :] = res.results[c]["logvar"]
    return mean, logv
